# revision 18
# baseline (speedup 1.0000x reference)
"""Trainium2 Bass kernel for nn_Encoder_Predictor (GRU-ish tree encoder with
per-step LayerNorm, action policy head, depth-4 stack).

Design notes:
  - Host-side (input-independent) weight folding:
      * depth-0 embedding folded into depth-0 projections (W0 = W_emb @ W etc)
      * LN mean comes out of the projection matmul via an appended mean column
      * LN sum-of-squares for the recurrent step via M = U @ U^T plus a
        row-dot with h (runs in parallel with the main matmul)
      * z-gate weight columns negated so (1 - z) is a plain sigmoid output
  - rsqrt for LN is a custom DVE op chain (bit-trick seed + 2 Newton steps),
    so ScalarE only ever uses the `sigmoid_and_others` table set
    (sigmoid/tanh/square/copy) -> zero ACT table switches.
  - The scan runs in chunks of CH steps; input projection (bulk LN), the
    action head, mask propagation and DRAM spills are per-chunk work that
    Tile overlaps with the sequential recurrence.
  - h state is kept both as (B, H) rows and, via per-step PE transposes, as
    hT (H, B) columns which feed the next step's matmuls and accumulate into
    the chunk buffer that becomes the next depth's input stream.
"""

from contextlib import ExitStack

import numpy as np

import concourse.bass as bass
import concourse.bacc as bacc_mod
import concourse.mybir as mybir
import concourse.tile as tile
from concourse.bass_utils import run_bass_kernel_spmd

B, L, D_IN, H, A, DEPTH = 64, 512, 256, 256, 128, 4
TH = 3 * H
EPS = 1e-5
F32 = mybir.dt.float32

QUAKE_K = float(np.float32(0x5F3759DF))
I32 = mybir.dt.int32

_OPS_READY = False
OP_VAR = OP_NR = None


def _register_custom_ops():
    global _OPS_READY, OP_VAR, OP_NR
    if _OPS_READY:
        return
    import concourse.dve_ops as dve_ops
    from concourse.dve_ops import DveOp
    from concourse.dve_spec import Spec, Src0, Src1, C0, C1, sq, lower, _has_src1
    from concourse.dve_uop import DveOpSpec

    def _ref_var(in0, in1, s0, s1, imm2):
        in1 = np.asarray(in1, np.float32)
        return ((in0.astype(np.float32) * np.float32(s0) - in1 * in1)
                + np.float32(s1)).astype(np.float32)

    def _ref_nr(in0, in1, s0, s1, imm2):
        v = in0.astype(np.float32)
        y = np.asarray(in1, np.float32)
        return (y * (np.float32(s1) - (y * y) * v * np.float32(s0))).astype(np.float32)

    defs = [
        ("ANT_RSQ_VAR", Spec(body=(Src0 * C0 - sq(Src1)) + C1, reference=_ref_var)),
        ("ANT_RSQ_NR", Spec(body=Src1 * (C1 - sq(Src1) * Src0 * C0),
                            reference=_ref_nr)),
    ]
    made = {}
    for name, spec in defs:
        ex = [o for o in dve_ops.OPS if o.name == name]
        if ex:
            made[name] = ex[0]
            continue
        op = DveOp(name, spec, subdim=False, uops_sha={})
        dve_ops.OPS.append(op)
        row = dve_ops._CUSTOM_DVE_ROW_BASE + len(dve_ops.OPS) - 1
        assert row < 0x20
        dve_ops._SUB_OPCODE_FOR_NAME[name] = row
        for ver in ("v3", "v4"):
            s = DveOpSpec(name=name, opcode=row, uops=lower(spec, ver=ver),
                          rd1_en=_has_src1(spec))
            op.uops_sha[ver] = s.sha(ver)
        dve_ops.CUSTOM_DVE_SPECS[name] = op.spec
        made[name] = op
    OP_VAR, OP_NR = made["ANT_RSQ_VAR"], made["ANT_RSQ_NR"]
    _OPS_READY = True


def _rsqrt_chain(nc, tiny, P, ssq_ap, mu_ap, n_nr, pfx):
    """invstd = rsqrt(ssq/TH - mu^2 + EPS), all on DVE.

    Seed is the Quake trick done in the value domain (the DVE's shift ops are
    dead on trn2 silicon): float(bits(v)) -> K - bits/2 -> bitcast back.
    """
    ve = tiny.tile([P, 1], F32, tag=pfx + "ve")
    nc.vector._custom_dve(OP_VAR, out=ve[:], in0=ssq_ap, in1=mu_ap,
                          s0=float(1.0 / TH), s1=float(EPS))
    vf = tiny.tile([P, 1], F32, tag=pfx + "vf")
    nc.vector.tensor_copy(vf[:], ve[:].bitcast(I32))
    jf = tiny.tile([P, 1], F32, tag=pfx + "jf")
    nc.vector.tensor_scalar(out=jf[:], in0=vf[:], scalar1=-0.5, scalar2=QUAKE_K,
                            op0=mybir.AluOpType.mult, op1=mybir.AluOpType.add)
    y0i = tiny.tile([P, 1], I32, tag=pfx + "y0")
    nc.vector.tensor_copy(y0i[:], jf[:])
    y = y0i[:].bitcast(F32)
    for i in range(n_nr):
        y2 = tiny.tile([P, 1], F32, tag=pfx + f"y{i+1}")
        nc.vector._custom_dve(OP_NR, out=y2[:], in0=ve[:], in1=y,
                              s0=0.5, s1=1.5)
        y = y2[:]
    return y


def _host_prep(inp):
    W_emb = inp["W_emb"].astype(np.float32); b_emb = inp["b_emb"].astype(np.float32)
    W = inp["W"].astype(np.float32); U = inp["U"].astype(np.float32)
    b = inp["b"].astype(np.float32)
    Wa1 = inp["W_action_1"].astype(np.float32); Ua1 = inp["U_action_1"].astype(np.float32)
    ba1 = inp["b_action_1"].astype(np.float32)
    Wa2 = inp["W_action_2"].astype(np.float32); ba2 = inp["b_action_2"].astype(np.float32)
    gam = inp["gammas"].astype(np.float32); bet = inp["betas"].astype(np.float32)
    assert np.all(gam == 1.0) and np.all(bet == 0.0) and np.all(b == 0.0), \
        "general gamma/beta/b path not implemented"
    assert np.all(b_emb == 0.0), "nonzero b_emb path not implemented"

    def aug(Wm):
        wm = Wm.mean(axis=1)
        return np.concatenate([Wm[:, :H], -Wm[:, H:2 * H], Wm[:, 2 * H:],
                               wm[:, None]], axis=1).astype(np.float32)

    W0 = (W_emb @ W).astype(np.float32)
    return {
        "Waug0": aug(W0), "Waug": aug(W), "Uaug": aug(U),
        "Msq": (U @ U.T).astype(np.float32),
        "Wa1_0": (W_emb @ Wa1).astype(np.float32), "Wa1": Wa1, "Ua1": Ua1,
        "wa2d": (Wa2[:, 1] - Wa2[:, 0]).reshape(A, 1).astype(np.float32),
        "ca0": (ba1 + b_emb @ Wa1).reshape(A, 1).astype(np.float32),
        "ca": ba1.reshape(A, 1).astype(np.float32),
        "cdiff": float(ba2[1] - ba2[0]),
        "I64": np.eye(64, dtype=np.float32),
    }


def _mega_layout(NTOK):
    return [("Waug0", TH + 1, 2), ("Waug", TH + 1, 2), ("Uaug", TH + 1, 2),
            ("Msq", H, 2), ("Wa1_0", A, 2), ("Wa1", A, 2), ("Ua1", A, 2),
            ("wa2d", 1, 1), ("ca0", 1, 1), ("ca", 1, 1), ("I64", 64, 1),
            ("cdiff", 1, 1), ("m_bulk0", NTOK // 128, 1)]


CHUNK = 8


def _build(CH=CHUNK, n_nr=2, depth=DEPTH, seq=L):
    _register_custom_ops()
    NCH = seq // CH
    NTOK = seq * B
    CTOK = CH * B
    NG = CTOK // 128
    assert CH % 2 == 0 and NG * 128 == CTOK and NCH * CH == seq

    nc = bacc_mod.Bacc()
    dp = lambda n, sh: nc.declare_dram_parameter(n, list(sh), F32, isOutput=False)
    xT = dp("xT", (2, 128, NTOK))
    layout = _mega_layout(NTOK)
    WMEGA = sum(w * k for _, w, k in layout)
    wmega = dp("wmega", (128, WMEGA))
    m_cell0 = dp("m_cell0", (B, seq))

    out_h = nc.declare_dram_parameter("out_h", [B, H], F32, isOutput=True)
    out_act = nc.declare_dram_parameter("out_act", [depth, NTOK], F32, isOutput=True)
    out_pol = nc.declare_dram_parameter("out_pol", [depth, NTOK], F32, isOutput=True)

    Sig = mybir.ActivationFunctionType.Sigmoid
    Tanh = mybir.ActivationFunctionType.Tanh
    Sq = mybir.ActivationFunctionType.Square
    Copy = mybir.ActivationFunctionType.Copy
    Alu = mybir.AluOpType

    with tile.TileContext(nc) as tc, ExitStack() as ctx:
        wpool = ctx.enter_context(tc.tile_pool(name="weights", bufs=1))
        inpool = ctx.enter_context(tc.tile_pool(name="inchunk", bufs=2))
        xpbpool = ctx.enter_context(tc.tile_pool(name="xpb", bufs=2))
        hbpool = ctx.enter_context(tc.tile_pool(name="hTbuf", bufs=2))
        mpool = ctx.enter_context(tc.tile_pool(name="masks", bufs=1))
        spool = ctx.enter_context(tc.tile_pool(name="scratch", bufs=3))
        tiny = ctx.enter_context(tc.tile_pool(name="tiny", bufs=6))
        actp = ctx.enter_context(tc.tile_pool(name="actsb", bufs=2))
        psZ = ctx.enter_context(tc.tile_pool(name="psZ", bufs=1, space="PSUM"))
        psG = ctx.enter_context(tc.tile_pool(name="psG", bufs=1, space="PSUM"))
        psT = ctx.enter_context(tc.tile_pool(name="psT", bufs=1, space="PSUM"))
        psB = ctx.enter_context(tc.tile_pool(name="psB", bufs=2, space="PSUM"))
        drp = ctx.enter_context(tc.tile_pool(name="spill", bufs=2, space="DRAM"))

        wmt = wpool.tile([128, WMEGA], F32, tag="wmega")
        nc.sync.dma_start(out=wmt[:], in_=wmega[:])
        wt = {}
        off = 0
        for nm, w, k in layout:
            wt[nm] = wmt[:, off:off + w * k]
            off += w * k
        wt["I64"] = wt["I64"][0:64, :]
        wt["cdiff"] = wt["cdiff"][0:1, :]

        def K(name, k, w):
            return wt[name][:, k * w:(k + 1) * w]

        # masks: cell layout (B, seq) and bulk layout (128, NTOK/128), 2 deep
        dmy0 = psT.tile([128, 128], F32, tag="tp", name="dmy0")
        nc.tensor.matmul(dmy0[0:1, 0:1], wmt[:, 0:1], wmt[:, 0:1],
                         start=True, stop=True)
        m_cell = [mpool.tile([B, seq], F32, tag=f"mc{i}", name=f"mc{i}")
                  for i in range(2)]
        m_celln = [mpool.tile([B, seq], F32, tag=f"mcn{i}", name=f"mcn{i}")
                   for i in range(2)]
        m_bulk = [mpool.tile([128, NTOK // 128], F32, tag=f"mb{i}", name=f"mb{i}")
                  for i in range(2)]
        nc.sync.dma_start(out=m_cell[0][:], in_=m_cell0[:])
        nc.sync.dma_start(out=m_bulk[0][:], in_=wt["m_bulk0"])
        nc.vector.tensor_scalar(out=m_celln[0][:], in0=m_cell[0][:],
                                scalar1=-1.0, scalar2=None, op0=Alu.mult)

        def bulk_chunk(d, in_k0, in_k1):
            # dummy 1x1 matmul so PE observes the input-chunk DMA semaphores
            # here; real matmuls then stay under the ISA sync-wait slot limit
            dmy = psT.tile([128, 128], F32, tag="tp", name="dmy")
            nc.tensor.matmul(dmy[0:1, 0:1], in_k0[:, 0:1], in_k1[:, 0:1],
                             start=True, stop=True)
            wa = "Waug0" if d == 0 else "Waug"
            xpbS_rz = xpbpool.tile([64, CH * 2 * H], F32, tag="xpbS_rz")
            xpbS_h = xpbpool.tile([64, CH * H], F32, tag="xpbS_h")
            for k in range(CH):
                # one step per matmul group (M=64) so every bulk tensor is
                # base-partition-0 and no cross-partition repack is needed
                zb = psB.tile([64, 1024], F32, tag="shared", name="zb")
                lhs0 = in_k0[:, k * 64:(k + 1) * 64]
                lhs1 = in_k1[:, k * 64:(k + 1) * 64]
                nc.tensor.matmul(zb[:, 0:512], lhs0, K(wa, 0, TH + 1)[:, 0:512],
                                 start=True, stop=False)
                nc.tensor.matmul(zb[:, 0:512], lhs1, K(wa, 1, TH + 1)[:, 0:512],
                                 start=False, stop=True)
                nc.tensor.matmul(zb[:, 512:769], lhs0, K(wa, 0, TH + 1)[:, 512:769],
                                 start=True, stop=False)
                nc.tensor.matmul(zb[:, 512:769], lhs1, K(wa, 1, TH + 1)[:, 512:769],
                                 start=False, stop=True)
                sqj = spool.tile([64, TH], F32, tag="sqj")
                ssq = tiny.tile([64, 1], F32, tag="bssq")
                nc.scalar.activation(sqj[:], zb[:, 0:TH], Sq, accum_out=ssq[:])
                mu_ap = zb[:, TH:TH + 1]
                inv = _rsqrt_chain(nc, tiny, 64, ssq[:], mu_ap, n_nr, "b")
                nmi = tiny.tile([64, 1], F32, tag="bnmi")
                nc.vector.scalar_tensor_tensor(out=nmi[:], in0=mu_ap, scalar=-1.0,
                                               in1=inv, op0=Alu.mult, op1=Alu.mult)
                pmi = tiny.tile([64, 1], F32, tag="bpmi")
                nc.vector.tensor_scalar(out=pmi[:], in0=nmi[:], scalar1=-1.0,
                                        scalar2=None, op0=Alu.mult)
                o = k * 2 * H
                nc.vector.tensor_scalar(out=xpbS_rz[:, o:o + H], in0=zb[:, 0:H],
                                        scalar1=inv, scalar2=nmi[:],
                                        op0=Alu.mult, op1=Alu.add)
                nc.vector.tensor_scalar(out=xpbS_rz[:, o + H:o + 2 * H],
                                        in0=zb[:, H:2 * H],
                                        scalar1=inv, scalar2=pmi[:],
                                        op0=Alu.mult, op1=Alu.add)
                nc.vector.tensor_scalar(out=xpbS_h[:, k * H:(k + 1) * H],
                                        in0=zb[:, 2 * H:TH],
                                        scalar1=inv, scalar2=nmi[:],
                                        op0=Alu.mult, op1=Alu.add)
            return xpbS_rz, xpbS_h

        prev_sp = None
        for d in range(depth):
            mcur, mnxt = m_cell[d % 2], m_cell[(d + 1) % 2]
            mcurn, mnxtn = m_celln[d % 2], m_celln[(d + 1) % 2]
            mbcur, mbnxt = m_bulk[d % 2], m_bulk[(d + 1) % 2]
            if d + 1 < depth:
                sp0 = drp.tile([128, NTOK], F32, tag="sp0")
                sp1 = drp.tile([128, NTOK], F32, tag="sp1")
            h = spool.tile([B, H], F32, tag="h")
            nc.vector.memset(h[:], 0.0)
            bk0 = hbpool.tile([128, (CH + 1) * 64], F32, tag="bk0")
            bk1 = hbpool.tile([128, (CH + 1) * 64], F32, tag="bk1")
            nc.vector.memset(bk0[:, 0:64], 0.0)
            nc.vector.memset(bk1[:, 0:64], 0.0)

            for c in range(NCH):
                tok0 = c * CTOK
                in_k0 = inpool.tile([128, CTOK], F32, tag="ink0")
                in_k1 = inpool.tile([128, CTOK], F32, tag="ink1")
                if d == 0:
                    nc.sync.dma_start(out=in_k0[:], in_=xT[0][:, tok0:tok0 + CTOK])
                    nc.sync.dma_start(out=in_k1[:], in_=xT[1][:, tok0:tok0 + CTOK])
                else:
                    nc.sync.dma_start(out=in_k0[:], in_=prev_sp[0][:, tok0:tok0 + CTOK])
                    nc.sync.dma_start(out=in_k1[:], in_=prev_sp[1][:, tok0:tok0 + CTOK])
                # mask chunk, bulk (token-linear) layout: (1, CTOK)
                m_in = actp.tile([1, CTOK], F32, tag="m_in")
                RPC = CTOK // (NTOK // 128)
                nc.sync.dma_start(out=m_in[:],
                                  in_=mbcur[c * RPC:(c + 1) * RPC, :])
                dvj = tiny.tile([1, 1], F32, tag="dvj", name="dvj")
                nc.vector.tensor_tensor(out=dvj[:], in0=m_in[0:1, 0:1],
                                        in1=in_k0[0:1, 0:1], op=Alu.add)
                xpb_rz, xpb_h = bulk_chunk(d, in_k0, in_k1)

                for k in range(CH):
                    t = c * CH + k
                    ph = (k % 2) * 64
                    grp = k // 2
                    zps = psZ.tile([B, 1024], F32, tag="z")
                    gps = psG.tile([B, H], F32, tag="g")
                    lhs0 = bk0[:, k * 64:(k + 1) * 64]
                    lhs1 = bk1[:, k * 64:(k + 1) * 64]
                    nc.tensor.matmul(gps[:], lhs0, K("Msq", 0, H), start=True, stop=False)
                    nc.tensor.matmul(gps[:], lhs1, K("Msq", 1, H), start=False, stop=True)
                    nc.tensor.matmul(zps[:, 512:769], lhs0,
                                     K("Uaug", 0, TH + 1)[:, 512:769], start=True, stop=False)
                    nc.tensor.matmul(zps[:, 512:769], lhs1,
                                     K("Uaug", 1, TH + 1)[:, 512:769], start=False, stop=True)
                    nc.tensor.matmul(zps[:, 0:512], lhs0,
                                     K("Uaug", 0, TH + 1)[:, 0:512], start=True, stop=False)
                    nc.tensor.matmul(zps[:, 0:512], lhs1,
                                     K("Uaug", 1, TH + 1)[:, 0:512], start=False, stop=True)

                    scr = spool.tile([B, H], F32, tag="scr")
                    ssq = tiny.tile([B, 1], F32, tag="ssq")
                    nc.vector.scalar_tensor_tensor(out=scr[:], in0=gps[:], scalar=1.0,
                                                   in1=h[:], op0=Alu.bypass,
                                                   op1=Alu.mult, accum_out=ssq[:])
                    mu_ap = zps[:, TH:TH + 1]
                    inv = _rsqrt_chain(nc, tiny, B, ssq[:], mu_ap, n_nr, "s")
                    nmi = tiny.tile([B, 1], F32, tag="nmi")
                    nc.vector.scalar_tensor_tensor(out=nmi[:], in0=mu_ap, scalar=-1.0,
                                                   in1=inv, op0=Alu.mult, op1=Alu.mult)
                    pmi = tiny.tile([B, 1], F32, tag="pmi")
                    nc.vector.tensor_scalar(out=pmi[:], in0=nmi[:], scalar1=-1.0,
                                            scalar2=None, op0=Alu.mult)
                    pre = spool.tile([B, 2 * H], F32, tag="pre")
                    nc.vector.affine_then_add(
                        out=pre[:], in0=zps[:, 0:2 * H],
                        in1=xpb_rz[:, k * 2 * H:(k + 1) * 2 * H],
                        scale=inv, bias=0.0)
                    r = spool.tile([B, H], F32, tag="r")
                    nc.scalar.activation(r[:], pre[:, 0:H], Sig, bias=nmi[:])
                    zc = spool.tile([B, H], F32, tag="zc")
                    nc.scalar.activation(zc[:], pre[:, H:2 * H], Sig, bias=pmi[:])
                    thh = spool.tile([B, H], F32, tag="thh")
                    jacc = tiny.tile([B, 1], F32, tag="jacc")
                    nc.vector.affine_mul_reduce(out=thh[:], accum_out=jacc[:],
                                                in0=zps[:, 2 * H:TH], in1=r[:],
                                                scale=inv, bias=nmi[:])
                    thh2 = spool.tile([B, H], F32, tag="thh2")
                    nc.vector.tensor_add(thh2[:], thh[:],
                                         xpb_h[:, k * H:(k + 1) * H])
                    hh = spool.tile([B, H], F32, tag="hh")
                    nc.scalar.activation(hh[:], thh2[:], Tanh)
                    p_ = spool.tile([B, H], F32, tag="p_")
                    nc.vector.tensor_mul(p_[:], zc[:], h[:])
                    a1 = spool.tile([B, H], F32, tag="a1")
                    nc.vector.affine_then_add(out=a1[:], in0=p_[:], in1=h[:],
                                              scale=mcurn[:, t:t + 1], bias=0.0)
                    mzchh = spool.tile([B, H], F32, tag="mzchh")
                    jacc2 = tiny.tile([B, 1], F32, tag="jacc2")
                    nc.vector.affine_mul_reduce(out=mzchh[:], accum_out=jacc2[:],
                                                in0=hh[:], in1=zc[:],
                                                scale=mcur[:, t:t + 1], bias=0.0)
                    h_new = spool.tile([B, H], F32, tag="h")
                    nc.vector.tensor_add(h_new[:], mzchh[:], a1[:])
                    tp = psT.tile([128, 128], F32, tag="tp")
                    nc.tensor.transpose(tp[:, 0:64], h_new[:, 0:128], wt["I64"])
                    nc.tensor.transpose(tp[:, 64:128], h_new[:, 128:256], wt["I64"])
                    nc.scalar.activation(bk0[:, (k + 1) * 64:(k + 2) * 64],
                                         tp[:, 0:64], Copy)
                    nc.scalar.activation(bk1[:, (k + 1) * 64:(k + 2) * 64],
                                         tp[:, 64:128], Copy)
                    h = h_new
                    if d == depth - 1 and t == seq - 1:
                        nc.sync.dma_start(out=out_h[:], in_=h_new[:])

                # ---- action head for the chunk (h_{t-1} = bk slots 0..CH-1)
                wa1 = "Wa1_0" if d == 0 else "Wa1"
                cab = wt["ca0"] if d == 0 else wt["ca"]
                aps = psB.tile([128, CTOK], F32, tag="shared")
                for nh in range(CTOK // 512):
                    s = slice(nh * 512, (nh + 1) * 512)
                    nc.tensor.matmul(aps[:, s], K(wa1, 0, A), in_k0[:, s],
                                     start=True, stop=False)
                    nc.tensor.matmul(aps[:, s], K(wa1, 1, A), in_k1[:, s],
                                     start=False, stop=False)
                    nc.tensor.matmul(aps[:, s], K("Ua1", 0, A),
                                     bk0[:, 0:CH * 64][:, s], start=False, stop=False)
                    nc.tensor.matmul(aps[:, s], K("Ua1", 1, A),
                                     bk1[:, 0:CH * 64][:, s], start=False, stop=True)
                tanhT = actp.tile([128, CTOK], F32, tag="tanhT")
                nc.scalar.activation(tanhT[:], aps[:], Tanh, bias=cab)
                dps = psB.tile([1, CTOK], F32, tag="shared")
                for nh in range(CTOK // 512):
                    s = slice(nh * 512, (nh + 1) * 512)
                    nc.tensor.matmul(dps[:, s], wt["wa2d"], tanhT[:, s],
                                     start=True, stop=True)
                pol1 = actp.tile([1, CTOK], F32, tag="pol1")
                nc.scalar.activation(pol1[:], dps[:], Sig, bias=wt["cdiff"])
                masked = actp.tile([1, CTOK], F32, tag="masked")
                nc.vector.scalar_tensor_tensor(out=masked[:], in0=dps[:],
                                               scalar=wt["cdiff"], in1=m_in[:],
                                               op0=Alu.add, op1=Alu.mult)
                actm = actp.tile([1, CTOK], F32, tag="actm")
                nc.vector.tensor_scalar(out=actm[:], in0=masked[:], scalar1=0.0,
                                        scalar2=None, op0=Alu.is_gt)
                nc.sync.dma_start(out=out_act[d:d + 1, tok0:tok0 + CTOK], in_=actm[:])
                nc.sync.dma_start(out=out_pol[d:d + 1, tok0:tok0 + CTOK], in_=pol1[:])
                if d + 1 < depth:
                    # next-depth masks: bulk layout + cell layout (b-major read)
                    nc.sync.dma_start(out=mbnxt[c * RPC:(c + 1) * RPC, :],
                                      in_=actm[:])
                    mscr = drp.tile([1, CTOK], F32, tag="mscr")
                    nc.sync.dma_start(out=mscr[:], in_=actm[:])
                    nc.sync.dma_start(
                        out=mnxt[:, c * CH:(c + 1) * CH],
                        in_=mscr[:].rearrange("o (t b) -> (o b) t", b=64))
                    nc.vector.tensor_scalar(out=mnxtn[:, c * CH:(c + 1) * CH],
                                            in0=mnxt[:, c * CH:(c + 1) * CH],
                                            scalar1=-1.0, scalar2=None, op0=Alu.mult)
                    nc.sync.dma_start(out=sp0[:, tok0:tok0 + CTOK],
                                      in_=bk0[:, 64:64 + CH * 64])
                    nc.sync.dma_start(out=sp1[:, tok0:tok0 + CTOK],
                                      in_=bk1[:, 64:64 + CH * 64])
                if c + 1 < NCH:
                    nbk0 = hbpool.tile([128, (CH + 1) * 64], F32, tag="bk0")
                    nbk1 = hbpool.tile([128, (CH + 1) * 64], F32, tag="bk1")
                    nc.scalar.activation(nbk0[:, 0:64],
                                         bk0[:, CH * 64:(CH + 1) * 64], Copy)
                    nc.scalar.activation(nbk1[:, 0:64],
                                         bk1[:, CH * 64:(CH + 1) * 64], Copy)
                    bk0, bk1 = nbk0, nbk1
            if d + 1 < depth:
                prev_sp = (sp0, sp1)
    nc.compile()
    return nc


_CACHE = {}


def _get_nc(CH, depth, seq):
    key = (CH, depth, seq)
    if key not in _CACHE:
        _CACHE[key] = _build(CH=CH, depth=depth, seq=seq)
    return _CACHE[key]


def _in_map(inp, seq=L):
    x = inp["x"].astype(np.float32)
    mask = np.asarray(inp["mask"]).astype(bool)
    p = _host_prep(inp)
    xTm = x.transpose(2, 1, 0).reshape(D_IN, seq * B)

    def ksplit(w):
        return np.ascontiguousarray(np.stack([w[:128], w[128:]], axis=0))

    vals = {
        "Waug0": p["Waug0"], "Waug": p["Waug"], "Uaug": p["Uaug"],
        "Msq": p["Msq"], "Wa1_0": p["Wa1_0"], "Wa1": p["Wa1"], "Ua1": p["Ua1"],
        "wa2d": p["wa2d"], "ca0": p["ca0"], "ca": p["ca"],
        "I64": np.concatenate([p["I64"], np.zeros((64, 64), np.float32)], 0),
        "cdiff": np.full((128, 1), p["cdiff"], np.float32),
        "m_bulk0": mask.T.flatten().reshape(128, seq * B // 128).astype(np.float32),
    }
    cols = []
    for nm, w, k in _mega_layout(seq * B):
        v = vals[nm].astype(np.float32)
        if k == 2:
            v = np.concatenate([v[:128], v[128:]], axis=1)
        assert v.shape == (128, w * k), (nm, v.shape, w, k)
        cols.append(v)
    return {
        "xT": ksplit(xTm),
        "wmega": np.ascontiguousarray(np.concatenate(cols, axis=1)),
        "m_cell0": np.ascontiguousarray(mask.astype(np.float32)),
    }


def kernel(**inputs):
    inp = {k: np.asarray(v) if not np.isscalar(v) else v for k, v in inputs.items()}
    depth = int(np.asarray(inp["depth"]))
    assert depth == DEPTH
    nc = _get_nc(CHUNK, depth, L)
    res = run_bass_kernel_spmd(nc, [_in_map(inp)], [0]).results[0]
    out_h = np.asarray(res["out_h"]).astype(np.float32)
    act = np.asarray(res["out_act"]).reshape(depth, L, B).transpose(2, 0, 1) > 0.5
    pol1 = np.asarray(res["out_pol"]).reshape(depth, L, B).transpose(2, 0, 1).astype(np.float32)
    policy = np.stack([(1.0 - pol1).astype(np.float32), pol1], axis=-1)
    return out_h, act, policy
